# revision 10
# baseline (speedup 1.0000x reference)
"""Dense 2-layer GAT forward on 8 Trainium2 NeuronCores.

Shapes (hardcoded): B=16 graphs, N=1024 nodes, F_IN=128, H=8 heads, D=64,
C=16 classes.  Data-parallel over batch: each of the 8 cores processes 2
full graphs with replicated (host-prefused) parameters.

Math notes:
  * f1 = X @ (W[h] @ a1)  -> fused into one "scores" matmul with
    V = [W@a1 | W@a2]  (shape [F, 2H]).
  * exp(leakyrelu(f1[i]+f2[j])) == max(E1[i]*E2[j], F1[i]*F2[j]) with
    E=exp(f), F=exp(0.2 f) -- exact, removes all N x N transcendentals.
  * Attention is built TRANSPOSED (pT[j, i]) so the attn @ Wh matmul needs
    no transposes of p; a ones-column appended to Wh gives the softmax
    denominator as a free extra PSUM row.
  * No max-subtraction in softmax: scores are O(1) so exp never overflows;
    identical math to the reference up to fp rounding.
  * elu(x) = min(exp(x) - 1, relu(x))  (exact).

Host <-> device traffic is the wall-clock bottleneck (the NeuronCores are
reached through a ~40 MB/s tunnel), so inputs are compressed host-side:
  * xs ships as fp16 (4 MB instead of 8),
  * adjacency ships bit-packed, 8 columns per byte (2 MB instead of 64) and
    is unpacked on-device with one AND + one is_gt per 128x1024 tile,
  * all parameters are pre-fused into a single small fp16 array.
The jitted SPMD executable is cached across calls; the first call goes
through bass_utils.run_bass_kernel_spmd (which re-traces/lowers on every
invocation), later calls reuse the cached executable so only input upload,
execution and the tiny output download remain.
"""

import os
import numpy as np

B, N, F_IN, H, D, C = 16, 1024, 128, 8, 64, 16
NCORES = 8
G = B // NCORES          # graphs per core = 2
ALPHA = 0.2
NT = N // 128            # 8 node chunks
HD = H * D               # 512
CCH = HD // 128          # 4 hd chunks
NB = N // 8              # 128 packed adjacency bytes per row

# fused parameter array layout (fp16, [128, P_COLS])
P_WALL = 0               # [:, 0:512]    W as [F_IN, H*D]
P_V = 512                # [:, 512:528]  [W@a1 | W@a2]
P_WO = 528               # [:, 528:600]  woaug [512,18] as [128, 4, 18]
P_WP = 600               # [0:16, 600:616] Wp
P_BP = 616               # [0:16, 616]   bp
P_COLS = 617

# Fraction of the 72 big (h,jc) tiles routed through the ScalarE
# (Lrelu+Exp) path instead of the VectorE (mul/mul/max) path.
ACT_TILES = int(os.environ.get("GAT_ACT_TILES", "52"))

_PROG = None
_FAST = None


def _route_is_act(idx, total=72, nact=None):
    if nact is None:
        nact = ACT_TILES
    return ((idx + 1) * nact) // total - (idx * nact) // total == 1


def _bcast_part(row_ap, parts):
    """[1, n] AP -> [parts, n] AP with partition step 0 (DMA source only)."""
    import concourse.bass as bass
    ap = [list(d) for d in row_ap.ap]
    return bass.AP(tensor=row_ap.tensor, offset=row_ap.offset,
                   ap=[[0, parts]] + ap[1:])


def _free_bcast(ap2, inner):
    """[P, k] AP -> [P, k, inner] AP with inner step 0 (compute-engine ok)."""
    import concourse.bass as bass
    ap = [list(d) for d in ap2.ap]
    return bass.AP(tensor=ap2.tensor, offset=ap2.offset, ap=ap + [[0, inner]])


def _build():
    import concourse.bass as bass
    import concourse.mybir as mybir
    from concourse import bacc
    from concourse.tile import TileContext
    from concourse.masks import make_identity

    f32 = mybir.dt.float32
    f16 = mybir.dt.float16
    u8 = mybir.dt.uint8
    AF = mybir.ActivationFunctionType
    OP = mybir.AluOpType

    nc = bacc.Bacc()

    xs_d = nc.dram_tensor("xs", [G, N, F_IN], f16, kind="ExternalInput")
    adjp_d = nc.dram_tensor("adjp", [G, N, NB], u8, kind="ExternalInput")
    pall_d = nc.dram_tensor("pall", [128, P_COLS], f16, kind="ExternalInput")
    out_d = nc.dram_tensor("out", [G, C], f32, kind="ExternalOutput")
    # DRAM scratch for partition-broadcast sources (slot: 0=esc 1=fsc
    # 2=raw-f1 3=layer2 rows)
    rs_d = nc.dram_tensor("rowscratch", [G, 4, 2 * H, N], f16)

    with TileContext(nc) as tc:
        with (
            tc.tile_pool(name="singles", bufs=1) as singles,
            tc.tile_pool(name="big1", bufs=1) as big1,
            tc.tile_pool(name="stage", bufs=3) as stage,
            tc.tile_pool(name="rows", bufs=1) as rows,
            tc.tile_pool(name="bcast", bufs=3) as bcast,
            tc.tile_pool(name="tmp", bufs=3) as tmp,
            tc.tile_pool(name="ptile", bufs=4) as ptile,
            tc.tile_pool(name="fin", bufs=2) as fin,
            tc.tile_pool(name="big2", bufs=2) as big2,
            tc.tile_pool(name="ps_wide", bufs=2, space="PSUM") as ps_wide,
            tc.tile_pool(name="ps_sq", bufs=2, space="PSUM") as ps_sq,
        ):
            # ---- constants / params -------------------------------------
            ident = singles.tile([128, 128], f32, tag="ident")
            make_identity(nc, ident[:])
            ident_h = singles.tile([128, 128], f16, tag="ident_h")
            make_identity(nc, ident_h[:])
            ones_col = singles.tile([128, 1], f32, tag="ones_col")
            nc.vector.memset(ones_col[:], 1.0)
            # bitmask tile for adjacency unpack: bmask[p, jb*8+t] = 1<<(7-t)
            bmask = singles.tile([128, N], u8, tag="bmask")
            bm3 = bmask[:].rearrange("p (a b) -> p a b", b=8)
            for t in range(8):
                nc.gpsimd.memset(bm3[:, :, t:t + 1], 1 << (7 - t))
            # Warm-up transposes: PE observes the identity writers (gpsimd)
            # here so every later transpose carries at most one wait
            # (walrus's PE wait-slot budget is tiny).
            ps_warm = ps_sq.tile([128, 128], f32, tag="sq")
            nc.tensor.transpose(out=ps_warm[:], in_=ident[:],
                                identity=ident[:])
            ps_warm2 = ps_sq.tile([128, 128], f16, tag="sq")
            nc.tensor.transpose(out=ps_warm2[:], in_=ident_h[:],
                                identity=ident_h[:])
            junk = singles.tile([128, 1], f32, tag="junk")
            nc.vector.tensor_copy(out=junk[:], in_=ps_warm[:, 0:1])
            nc.vector.tensor_copy(out=junk[:], in_=ps_warm2[:, 0:1])

            pall_sb = singles.tile([128, P_COLS], f16, tag="pall")
            nc.scalar.dma_start(out=pall_sb[:], in_=pall_d[:, :])
            wall_sb = pall_sb[:, P_WALL:P_WALL + HD]
            v_sb = pall_sb[:, P_V:P_V + 2 * H]
            woaug_sb = pall_sb[:, P_WO:P_WO + 72].rearrange(
                "p (c k) -> p c k", k=18)
            wp_sb = pall_sb[0:C, P_WP:P_WP + C]
            bp_f32 = singles.tile([C, 1], f32, tag="bp32")
            nc.vector.tensor_copy(out=bp_f32[:],
                                  in_=pall_sb[0:C, P_BP:P_BP + 1])

            for g in range(G):
                # ==== stage A: X load + transpose ========================
                xt_sb = big1.tile([128, N], f16, tag="xt")
                for nt in range(NT):
                    xtile = stage.tile([128, F_IN], f16, tag="xtile")
                    nc.scalar.dma_start(
                        out=xtile[:],
                        in_=xs_d[g, nt * 128:(nt + 1) * 128, :])
                    xtile2 = stage.tile([128, F_IN], f16, tag="xtile2")
                    nc.vector.tensor_copy(out=xtile2[:], in_=xtile[:])
                    ps_x = ps_sq.tile([128, 128], f16, tag="sq")
                    nc.tensor.transpose(out=ps_x[:], in_=xtile2[:],
                                        identity=ident_h[:])
                    nc.vector.tensor_copy(
                        out=xt_sb[:, nt * 128:(nt + 1) * 128], in_=ps_x[:])

                # ==== stage B: projection + whaug ========================
                whaug = big1.tile([128, NT, 8 * 65], f16, tag="whaug")
                for nt in range(NT):
                    ps_p = ps_sq.tile([128, HD], f32, tag="sq")
                    nc.tensor.matmul(
                        out=ps_p[:],
                        lhsT=xt_sb[:, nt * 128:(nt + 1) * 128],
                        rhs=wall_sb, start=True, stop=True)
                    w_slice = whaug[:, nt, :].rearrange(
                        "p (h c) -> p h c", c=65)
                    nc.vector.tensor_copy(
                        out=w_slice[:, :, 0:64],
                        in_=ps_p[:].rearrange("p (h c) -> p h c", c=64))
                    nc.gpsimd.memset(w_slice[:, :, 64:65], 1.0)

                # ==== stage: scores ======================================
                ps_sc = ps_wide.tile([2 * H, N], f32, tag="wide")
                for ih in range(2):
                    nc.tensor.matmul(
                        out=ps_sc[:, ih * 512:(ih + 1) * 512],
                        lhsT=v_sb,
                        rhs=xt_sb[:, ih * 512:(ih + 1) * 512],
                        start=True, stop=True)
                scores = rows.tile([2 * H, N], f32, tag="scores")
                nc.vector.tensor_copy(out=scores[:], in_=ps_sc[:])
                esc = rows.tile([2 * H, N], f16, tag="esc")
                nc.scalar.activation(esc[:], scores[:], AF.Exp)
                fsc = rows.tile([2 * H, N], f16, tag="fsc")
                nc.scalar.activation(fsc[:], scores[:], AF.Exp, scale=ALPHA)
                fsc_bf = rows.tile([2 * H, N], f16, tag="fscbf")
                nc.scalar.copy(out=fsc_bf[:], in_=scores[:])

                # transposed score columns + their exps
                scT = rows.tile([128, NT, 2 * H], f32, tag="scT")
                ecT = rows.tile([128, NT, 2 * H], f32, tag="ecT")
                fcT = rows.tile([128, NT, 2 * H], f32, tag="fcT")
                for jc in range(NT):
                    ps_t = ps_sq.tile([128, 2 * H], f32, tag="sq")
                    nc.tensor.transpose(
                        out=ps_t[:],
                        in_=scores[:, jc * 128:(jc + 1) * 128],
                        identity=ident[0:2 * H, 0:2 * H])
                    nc.vector.tensor_copy(out=scT[:, jc, :], in_=ps_t[:])
                    nc.scalar.activation(ecT[:, jc, :], scT[:, jc, :], AF.Exp)
                    nc.scalar.activation(fcT[:, jc, :], scT[:, jc, :], AF.Exp,
                                         scale=ALPHA)

                sc02 = rows.tile([128, NT, 2 * H], f32, tag="sc02")
                nc.vector.tensor_scalar(
                    out=sc02[:], in0=scT[:], scalar1=ALPHA, scalar2=None,
                    op0=OP.mult)

                # ==== stage C: row broadcasts (via DRAM bounce) ==========
                nc.scalar.dma_start(out=rs_d[g, 0, :, :], in_=esc[:])
                nc.scalar.dma_start(out=rs_d[g, 1, :, :], in_=fsc[:])
                nc.scalar.dma_start(out=rs_d[g, 2, :, :], in_=fsc_bf[:])
                e1b, f1b, l1b = [], [], []
                for h in range(H):
                    t_e = bcast.tile([128, N], f16, tag="e1b")
                    nc.scalar.dma_start(
                        out=t_e[:],
                        in_=_bcast_part(rs_d[g, 0, h:h + 1, :], 128))
                    t_f = bcast.tile([128, N], f16, tag="f1b")
                    nc.scalar.dma_start(
                        out=t_f[:],
                        in_=_bcast_part(rs_d[g, 1, h:h + 1, :], 128))
                    t_l = bcast.tile([128, N], f16, tag="l1b")
                    nc.scalar.dma_start(
                        out=t_l[:],
                        in_=_bcast_part(rs_d[g, 2, h:h + 1, :], 128))
                    e1b.append(t_e)
                    f1b.append(t_f)
                    l1b.append(t_l)

                # ==== stage D: adjacency unpack -> transposed ============
                # bytes hold 8 adjacency columns each (big bit order); AND
                # against the per-column bit mask then compare >0 to get
                # {0,1} fp16; PE transposes 128x128 blocks.
                adjT = big2.tile([128, NT, N], f16, tag="adjT")
                for it in range(NT):
                    adj_p = stage.tile([128, NB], u8, tag="adjp")
                    nc.scalar.dma_start(
                        out=adj_p[:],
                        in_=adjp_d[g, it * 128:(it + 1) * 128, :])
                    adj_an = stage.tile([128, N], u8, tag="adjan")
                    nc.vector.tensor_tensor(
                        out=adj_an[:].rearrange("p (a b) -> p a b", b=8),
                        in0=_free_bcast(adj_p[:], 8),
                        in1=bmask[:].rearrange("p (a b) -> p a b", b=8),
                        op=OP.bitwise_and)
                    adj_h = stage.tile([128, N], f16, tag="adjbf")
                    nc.vector.tensor_scalar(
                        out=adj_h[:], in0=adj_an[:], scalar1=0, scalar2=None,
                        op0=OP.is_gt)
                    ps_at = ps_sq.tile([128, N], f16, tag="sq")
                    for jc in range(NT):
                        nc.tensor.transpose(
                            out=ps_at[:, jc * 128:(jc + 1) * 128],
                            in_=adj_h[:, jc * 128:(jc + 1) * 128],
                            identity=ident_h[:])
                    nc.vector.tensor_copy(
                        out=adjT[:, :, it * 128:(it + 1) * 128],
                        in_=ps_at[:].rearrange("p (c i) -> p c i", i=128))

                # ==== stage E: attention layer 1 =========================
                oT = big1.tile([65, H, N], f32, tag="oT")
                for h in range(H):
                    ps_o = ps_wide.tile([65, N], f32, tag="wide")
                    for jc in range(NT):
                        pt = ptile.tile([128, N], f16, tag="pt")

                        if _route_is_act(h * NT + jc):
                            t_p1 = tmp.tile([128, N], f16, tag="tmp1")
                            nc.scalar.activation(
                                t_p1[:], l1b[h][:], AF.Exp,
                                bias=scT[:, jc, H + h:H + h + 1])
                            t_p2 = tmp.tile([128, N], f16, tag="tmp2")
                            nc.scalar.activation(
                                t_p2[:], l1b[h][:], AF.Exp, scale=ALPHA,
                                bias=sc02[:, jc, H + h:H + h + 1])
                            t_m = tmp.tile([128, N], f16, tag="tmp3")
                            nc.vector.tensor_tensor(
                                out=t_m[:], in0=t_p1[:], in1=t_p2[:],
                                op=OP.max)
                            nc.vector.tensor_tensor(
                                out=pt[:], in0=t_m[:], in1=adjT[:, jc, :],
                                op=OP.mult)
                        else:
                            t_a = tmp.tile([128, N], f16, tag="tmp1")
                            nc.vector.tensor_scalar(
                                out=t_a[:], in0=e1b[h][:],
                                scalar1=ecT[:, jc, H + h:H + h + 1], scalar2=None,
                                op0=OP.mult)
                            t_b = tmp.tile([128, N], f16, tag="tmp2")
                            nc.vector.tensor_scalar(
                                out=t_b[:], in0=f1b[h][:],
                                scalar1=fcT[:, jc, H + h:H + h + 1], scalar2=None,
                                op0=OP.mult)
                            t_m = tmp.tile([128, N], f16, tag="tmp3")
                            nc.vector.tensor_tensor(
                                out=t_m[:], in0=t_a[:], in1=t_b[:],
                                op=OP.max)
                            nc.gpsimd.tensor_tensor(
                                out=pt[:], in0=t_m[:], in1=adjT[:, jc, :],
                                op=OP.mult)

                        for ih in range(2):
                            nc.tensor.matmul(
                                out=ps_o[:, ih * 512:(ih + 1) * 512],
                                lhsT=whaug[:, jc, h * 65:(h + 1) * 65],
                                rhs=pt[:, ih * 512:(ih + 1) * 512],
                                start=(jc == 0), stop=(jc == NT - 1))
                    nc.vector.tensor_copy(out=oT[:, h, :], in_=ps_o[:])

                # ==== stage F: normalize + elu -> x1T (f16) ==============
                x1t = big1.tile([128, CCH, N], f16, tag="x1t")
                for it in range(NT):
                    # two 1-bank PSUM tiles (4 heads each): a [*, 65] block
                    # must never cross the 512-float bank boundary
                    ps_on_l = []
                    for half in range(2):
                        ps_on = ps_sq.tile([128, 4 * 65], f32, tag="sq")
                        for hh in range(4):
                            h = half * 4 + hh
                            nc.tensor.transpose(
                                out=ps_on[:, hh * 65:(hh + 1) * 65],
                                in_=oT[:, h, it * 128:(it + 1) * 128],
                                identity=ident[0:65, 0:65])
                        ps_on_l.append(ps_on)
                    rc = fin.tile([128, H], f32, tag="rc")
                    z = fin.tile([128, HD], f16, tag="z")
                    for half in range(2):
                        on3 = ps_on_l[half][:].rearrange(
                            "p (h c) -> p h c", c=65)
                        nc.vector.reciprocal(
                            out=rc[:, 4 * half:4 * half + 4, None],
                            in_=on3[:, :, 64:65])
                        nc.vector.tensor_tensor(
                            out=z[:, 256 * half:256 * half + 256].rearrange(
                                "p (h c) -> p h c", c=64),
                            in0=on3[:, :, 0:64],
                            in1=_free_bcast(rc[:, 4 * half:4 * half + 4], 64),
                            op=OP.mult)
                    ee = fin.tile([128, HD], f16, tag="ee")
                    nc.scalar.activation(ee[:], z[:], AF.Exp)
                    em1 = fin.tile([128, HD], f16, tag="em1")
                    nc.vector.tensor_scalar(
                        out=em1[:], in0=ee[:], scalar1=1.0, scalar2=None,
                        op0=OP.subtract)
                    rl = fin.tile([128, HD], f16, tag="rl")
                    nc.scalar.activation(rl[:], z[:], AF.Relu)
                    x1n = fin.tile([128, HD], f16, tag="x1n")
                    nc.vector.tensor_tensor(out=x1n[:], in0=em1[:],
                                            in1=rl[:], op=OP.min)
                    ps_xt = ps_sq.tile([128, HD], f16, tag="sq")
                    for cc in range(CCH):
                        nc.tensor.transpose(
                            out=ps_xt[:, cc * 128:(cc + 1) * 128],
                            in_=x1n[:, cc * 128:(cc + 1) * 128],
                            identity=ident_h[:])
                    nc.vector.tensor_copy(
                        out=x1t[:, :, it * 128:(it + 1) * 128],
                        in_=ps_xt[:].rearrange("p (c i) -> p c i", i=128))

                # ==== stage G: layer 2 ===================================
                ps_s2 = ps_wide.tile([18, N], f32, tag="wide")
                for cc in range(CCH):
                    for ih in range(2):
                        nc.tensor.matmul(
                            out=ps_s2[:, ih * 512:(ih + 1) * 512],
                            lhsT=woaug_sb[:, cc, :],
                            rhs=x1t[:, cc, ih * 512:(ih + 1) * 512],
                            start=(cc == 0), stop=(cc == CCH - 1))
                s2T = rows.tile([18, N], f32, tag="s2T")
                nc.vector.tensor_copy(out=s2T[:], in_=ps_s2[:])

                e1o = rows.tile([1, N], f16, tag="e1o")
                nc.scalar.activation(e1o[:], s2T[0:1, :], AF.Exp)
                f1o = rows.tile([1, N], f16, tag="f1o")
                nc.scalar.activation(f1o[:], s2T[0:1, :], AF.Exp,
                                     scale=ALPHA)
                l1o = rows.tile([1, N], f16, tag="l1o")
                nc.scalar.copy(out=l1o[:], in_=s2T[0:1, :])
                nc.scalar.dma_start(out=rs_d[g, 3, 0:1, :], in_=e1o[:])
                nc.scalar.dma_start(out=rs_d[g, 3, 1:2, :], in_=f1o[:])
                nc.scalar.dma_start(out=rs_d[g, 3, 2:3, :], in_=l1o[:])
                e1ob = bcast.tile([128, N], f16, tag="e1b")
                nc.scalar.dma_start(out=e1ob[:],
                                  in_=_bcast_part(rs_d[g, 3, 0:1, :], 128))
                f1ob = bcast.tile([128, N], f16, tag="f1b")
                nc.scalar.dma_start(out=f1ob[:],
                                  in_=_bcast_part(rs_d[g, 3, 1:2, :], 128))
                l1ob = bcast.tile([128, N], f16, tag="l1b")
                nc.scalar.dma_start(out=l1ob[:],
                                  in_=_bcast_part(rs_d[g, 3, 2:3, :], 128))

                wh2n = rows.tile([128, NT, 17], f16, tag="wh2n")
                w2all = rows.tile([128, NT, 18], f32, tag="w2all")
                w2s02 = rows.tile([128, NT, 1], f32, tag="w2s02")
                ec2c = rows.tile([128, NT, 1], f32, tag="ec2c")
                fc2c = rows.tile([128, NT, 1], f32, tag="fc2c")
                for jc in range(NT):
                    ps_w2 = ps_sq.tile([128, 18], f32, tag="sq")
                    nc.tensor.transpose(
                        out=ps_w2[:],
                        in_=s2T[:, jc * 128:(jc + 1) * 128],
                        identity=ident[0:18, 0:18])
                    nc.vector.tensor_copy(out=w2all[:, jc, :], in_=ps_w2[:])
                    nc.vector.tensor_copy(out=wh2n[:, jc, 0:16],
                                          in_=w2all[:, jc, 2:18])
                    nc.gpsimd.memset(wh2n[:, jc, 16:17], 1.0)
                    nc.vector.tensor_scalar(
                        out=w2s02[:, jc, :], in0=w2all[:, jc, 1:2],
                        scalar1=ALPHA, scalar2=None, op0=OP.mult)
                    nc.scalar.activation(ec2c[:, jc, :], w2all[:, jc, 1:2],
                                         AF.Exp)
                    nc.scalar.activation(fc2c[:, jc, :], w2all[:, jc, 1:2],
                                         AF.Exp, scale=ALPHA)

                ps_o2 = ps_wide.tile([17, N], f32, tag="wide")
                for jc in range(NT):
                    pt = ptile.tile([128, N], f16, tag="pt")
                    if _route_is_act(64 + jc):
                        t_p1 = tmp.tile([128, N], f16, tag="tmp1")
                        nc.scalar.activation(
                            t_p1[:], l1ob[:], AF.Exp,
                            bias=w2all[:, jc, 1:2])
                        t_p2 = tmp.tile([128, N], f16, tag="tmp2")
                        nc.scalar.activation(
                            t_p2[:], l1ob[:], AF.Exp, scale=ALPHA,
                            bias=w2s02[:, jc, 0:1])
                        t_m = tmp.tile([128, N], f16, tag="tmp3")
                        nc.vector.tensor_tensor(
                            out=t_m[:], in0=t_p1[:], in1=t_p2[:], op=OP.max)
                        nc.vector.tensor_tensor(
                            out=pt[:], in0=t_m[:], in1=adjT[:, jc, :],
                            op=OP.mult)
                    else:
                        t_a = tmp.tile([128, N], f16, tag="tmp1")
                        nc.vector.tensor_scalar(
                            out=t_a[:], in0=e1ob[:],
                            scalar1=ec2c[:, jc, 0:1], scalar2=None,
                            op0=OP.mult)
                        t_b = tmp.tile([128, N], f16, tag="tmp2")
                        nc.vector.tensor_scalar(
                            out=t_b[:], in0=f1ob[:],
                            scalar1=fc2c[:, jc, 0:1], scalar2=None,
                            op0=OP.mult)
                        t_m = tmp.tile([128, N], f16, tag="tmp3")
                        nc.vector.tensor_tensor(
                            out=t_m[:], in0=t_a[:], in1=t_b[:], op=OP.max)
                        nc.gpsimd.tensor_tensor(
                            out=pt[:], in0=t_m[:], in1=adjT[:, jc, :],
                            op=OP.mult)
                    for ih in range(2):
                        nc.tensor.matmul(
                            out=ps_o2[:, ih * 512:(ih + 1) * 512],
                            lhsT=wh2n[:, jc, :],
                            rhs=pt[:, ih * 512:(ih + 1) * 512],
                            start=(jc == 0), stop=(jc == NT - 1))
                o2T = rows.tile([17, N], f32, tag="o2T")
                nc.vector.tensor_copy(out=o2T[:], in_=ps_o2[:])

                # ==== stage H: normalize/elu layer 2 + mean + head =======
                ps_sum = ps_sq.tile([C, 1], f32, tag="sq")
                for it in range(NT):
                    ps_o2n = ps_sq.tile([128, 17], f32, tag="sq")
                    nc.tensor.transpose(
                        out=ps_o2n[:],
                        in_=o2T[:, it * 128:(it + 1) * 128],
                        identity=ident[0:17, 0:17])
                    rc2 = fin.tile([128, 1], f32, tag="rc2")
                    nc.vector.reciprocal(out=rc2[:], in_=ps_o2n[:, 16:17])
                    z2 = fin.tile([128, C], f32, tag="z2")
                    nc.vector.tensor_scalar(
                        out=z2[:], in0=ps_o2n[:, 0:16], scalar1=rc2[:, 0:1],
                        scalar2=None, op0=OP.mult)
                    ee2 = fin.tile([128, C], f32, tag="ee2")
                    nc.scalar.activation(ee2[:], z2[:], AF.Exp)
                    em2 = fin.tile([128, C], f32, tag="em2")
                    nc.vector.tensor_scalar(
                        out=em2[:], in0=ee2[:], scalar1=1.0, scalar2=None,
                        op0=OP.subtract)
                    rl2 = fin.tile([128, C], f32, tag="rl2")
                    nc.scalar.activation(rl2[:], z2[:], AF.Relu)
                    x2n = fin.tile([128, C], f32, tag="x2n")
                    nc.vector.tensor_tensor(out=x2n[:], in0=em2[:],
                                            in1=rl2[:], op=OP.min)
                    nc.tensor.matmul(
                        out=ps_sum[:], lhsT=x2n[:], rhs=ones_col[:],
                        start=(it == 0), stop=(it == NT - 1))
                ssum = fin.tile([C, 1], f16, tag="ssum")
                nc.vector.tensor_copy(out=ssum[:], in_=ps_sum[:])
                ps_pred = ps_sq.tile([C, 1], f32, tag="sq")
                nc.tensor.matmul(out=ps_pred[:], lhsT=wp_sb,
                                 rhs=ssum[:], start=True, stop=True)
                pred = fin.tile([C, 1], f32, tag="pred")
                nc.vector.tensor_scalar(
                    out=pred[:], in0=ps_pred[:], scalar1=1.0 / N,
                    scalar2=bp_f32[:], op0=OP.mult, op1=OP.add)
                nc.scalar.dma_start(out=out_d[g, :], in_=pred[:, 0:1])

    nc.compile()
    return nc


def _get_prog():
    global _PROG
    if _PROG is None:
        _PROG = _build()
    return _PROG


def _prep_global(xs, adjs, W, a1, a2, Wo, ao1, ao2, Wp, bp):
    """Host-side packing. Returns the three global (concatenated-over-core)
    input arrays keyed by BIR tensor name."""
    xs = np.asarray(xs)
    adjs = np.asarray(adjs)
    W = np.asarray(W, dtype=np.float32)
    a1 = np.asarray(a1, dtype=np.float32)
    a2 = np.asarray(a2, dtype=np.float32)
    Wo = np.asarray(Wo, dtype=np.float32)
    ao1 = np.asarray(ao1, dtype=np.float32)
    ao2 = np.asarray(ao2, dtype=np.float32)
    Wp = np.asarray(Wp, dtype=np.float32)
    bp = np.asarray(bp, dtype=np.float32)

    xs16 = np.ascontiguousarray(xs.astype(np.float16))
    # bit-pack adjacency; np.packbits releases the GIL so slice over threads
    adjp = np.empty((B, N, NB), np.uint8)
    from concurrent.futures import ThreadPoolExecutor

    def _pack(b0):
        adjp[b0:b0 + 4] = np.packbits(adjs[b0:b0 + 4].astype(np.uint8),
                                      axis=-1)
    with ThreadPoolExecutor(max_workers=4) as ex:
        list(ex.map(_pack, range(0, B, 4)))

    pall = np.zeros((128, P_COLS), np.float16)
    pall[:, P_WALL:P_WALL + HD] = W.transpose(1, 0, 2).reshape(F_IN, HD)
    pall[:, P_V:P_V + H] = np.einsum("hfd,hd->fh", W, a1)
    pall[:, P_V + H:P_V + 2 * H] = np.einsum("hfd,hd->fh", W, a2)
    woaug = np.concatenate(
        [(Wo @ ao1)[:, None], (Wo @ ao2)[:, None], Wo], axis=1)  # [512, 18]
    pall[:, P_WO:P_WO + 72] = woaug.reshape(
        CCH, 128, 18).transpose(1, 0, 2).reshape(128, 72)
    pall[0:C, P_WP:P_WP + C] = Wp
    pall[0:C, P_BP] = bp
    return {"xs": xs16, "adjp": adjp, "pall": pall}


def _build_fast(nc):
    """Build the cached jitted SPMD executable (the same mechanics as
    bass_utils.run_bass_kernel_spmd's axon path, minus the per-call
    re-trace/re-lower)."""
    import jax
    from jax.sharding import Mesh, PartitionSpec
    from jax.experimental.shard_map import shard_map
    import concourse.mybir as mybir
    from concourse.bass2jax import (_bass_exec_p, install_neuronx_cc_hook,
                                    partition_id_tensor)

    install_neuronx_cc_hook()

    partition_name = (nc.partition_id_tensor.name
                      if nc.partition_id_tensor else None)
    in_names, out_names, out_avals, zero_shapes = [], [], [], []
    for alloc in nc.m.functions[0].allocations:
        if not isinstance(alloc, mybir.MemoryLocationSet):
            continue
        name = alloc.memorylocations[0].name
        if alloc.kind == "ExternalInput":
            if name != partition_name:
                in_names.append(name)
        elif alloc.kind == "ExternalOutput":
            shape = tuple(alloc.tensor_shape)
            dtype = mybir.dt.np(alloc.dtype)
            out_avals.append(jax.core.ShapedArray(shape, dtype))
            out_names.append(name)
            zero_shapes.append((shape, dtype))
    n_params = len(in_names)
    n_outs = len(out_avals)
    in_names_full = list(in_names) + list(out_names)
    if partition_name is not None:
        in_names_full.append(partition_name)
    donate = tuple(range(n_params, n_params + n_outs))

    def _body(*args):
        operands = list(args)
        if partition_name is not None:
            operands.append(partition_id_tensor())
        outs = _bass_exec_p.bind(
            *operands,
            out_avals=tuple(out_avals),
            in_names=tuple(in_names_full),
            out_names=tuple(out_names),
            lowering_input_output_aliases=(),
            sim_require_finite=True,
            sim_require_nnan=True,
            nc=nc,
        )
        return tuple(outs)

    devices = jax.devices()[:NCORES]
    assert len(devices) == NCORES
    mesh = Mesh(np.asarray(devices), ("core",))
    # params are identical on every core -> replicate instead of shipping a
    # pre-tiled copy
    in_specs = tuple(
        PartitionSpec() if name == "pall" else PartitionSpec("core")
        for name in in_names) + (PartitionSpec("core"),) * n_outs
    out_specs = (PartitionSpec("core"),) * len(out_names)
    sharded = jax.jit(
        shard_map(_body, mesh=mesh, in_specs=in_specs, out_specs=out_specs,
                  check_rep=False),
        donate_argnums=donate,
        keep_unused=True,
    )

    def run(global_in: dict):
        args = [global_in[name] for name in in_names]
        zeros = [np.zeros((NCORES * s[0], *s[1:]), d)
                 for (s, d) in zero_shapes]
        out_arrs = sharded(*args, *zeros)
        # fetch the 8 tiny output shards in parallel
        arr = out_arrs[0]
        shards = arr.addressable_shards
        for s in shards:
            s.data.copy_to_host_async()
        out = np.empty(arr.shape, arr.dtype)
        for s in shards:
            out[s.index] = np.asarray(s.data)
        return out

    run.sharded = sharded
    run.in_names = in_names
    run.zero_shapes = zero_shapes
    run.mesh = mesh
    return run


def _get_fast():
    global _FAST
    if _FAST is None:
        _FAST = _build_fast(_get_prog())
    return _FAST


def _run_spmd_once(global_in):
    """The documented path: bass_utils.run_bass_kernel_spmd over cores 0-7.
    Used on the first invocation (it re-traces and re-lowers the module on
    every call, so repeat calls use the cached executable instead)."""
    from concourse.bass_utils import run_bass_kernel_spmd
    nc = _get_prog()
    in_maps = [
        {"xs": global_in["xs"][c * G:(c + 1) * G],
         "adjp": global_in["adjp"][c * G:(c + 1) * G],
         "pall": global_in["pall"]}
        for c in range(NCORES)
    ]
    res = run_bass_kernel_spmd(nc, in_maps, core_ids=list(range(NCORES)),
                               trace=False)
    out = np.concatenate([res.results[c]["out"] for c in range(NCORES)],
                         axis=0)
    return out, res


_FIRST_DONE = False


def _run(trace=False, **inputs):
    global _FIRST_DONE
    global_in = _prep_global(**inputs)
    if not _FIRST_DONE:
        out, res = _run_spmd_once(global_in)
        fast = _get_fast()
        out2 = fast(global_in)  # warm the cached executable
        _FIRST_DONE = True
        return out2, res
    fast = _get_fast()
    out = fast(global_in)
    return out, _NoRes()


class _NoRes:
    exec_time_ns = None
    results = None


def kernel(**inputs):
    out, _ = _run(trace=False, **inputs)
    return out
